# revision 12
# baseline (speedup 1.0000x reference)
"""Fused MHA Bass kernel for Trainium2, batch-parallel over 8 cores, fp8 DoubleRow.

Reference (per batch element):
    qkv = x @ w_qkv + b_qkv ; q,k,v = split(qkv)
    s = q @ k.T / 8 ; a = softmax(s) ; y = (a @ v) @ w_out + b_out

Math restructuring (exact algebra, host-folded weights):
    s*8 = x M x^T + (x wq bk) 1^T + 1 (x wk bq)^T + bq.bk,  M = wq wk^T
    y   = (a_unnorm @ u) / den + (b_out + bv w_out),        u = x (wv w_out)
so the k-projection, v-projection output and the final projection fold into
two [768,768] device matmuls (q' = x@M, u = x@N) and the attention itself.
The attention output lands directly in [tq, dy] layout: the softmax
denominator is an appended ones-column of u (per-partition => direct DVE
reciprocal, no transposes), and 1/den is applied at the y eviction.

Precision: every matmul is fp8 DoubleRow (0.5 cycles/row, 2 k-tiles per
instruction). Accuracy is recovered with hi+lo fp8 splits:
  x = xh + xl (e4m3),  16M = Mh + Ml,  16N = Nh + Nl  (host, residual split)
  16q' -> qh + ql (e4m3, on-device resplit of the PSUM)
  scores = qh@xh + qh@xl + ql@xh   (3 of 4 cross terms; lo@lo dropped)
  u -> uh + ul (e4m3);  exp(s-12) -> ehi (e5m2) + elo (e4m3)
  y_unnorm = ehi@uh + elo@uh + ehi@ul
Simulated end-to-end rel err vs fp32 reference: 7.1e-3 (gate 2e-2).
"""

import numpy as np
import ml_dtypes

import concourse.bacc as bacc
import concourse.bass as bass
import concourse.mybir as mybir
import concourse.tile as tile
from concourse import bass_utils

F32 = mybir.dt.float32
F8E4 = mybir.dt.float8e4
F8E5 = mybir.dt.float8e5
AF = mybir.ActivationFunctionType
DR = mybir.MatmulPerfMode.DoubleRow

B = 8
T = 2048
D = 768
ND = D // 128           # 6 d-tiles
NT = T // 128           # 16 t-tiles
TQB = 512               # query-block width
NBLK = T // TQB         # 4 blocks
UW = D + 16             # u width (col D = 16.0 denominator column, rest pad;
                        # 784 = 16*49 keeps the DoubleRow pair stride 16B-aligned)
CSH = 12.0              # global exp shift: exp(s - 12), e5m2 range covers it
E4NP = ml_dtypes.float8_e4m3
E5NP = ml_dtypes.float8_e5m2


def _build_program(nc, reps=1):
    xh_d = nc.dram_tensor("xt_h", [D, T], F8E4, kind="ExternalInput").ap()
    xl_d = nc.dram_tensor("xt_l", [D, T], F8E4, kind="ExternalInput").ap()
    mh_d = nc.dram_tensor("m_h", [D, D], F8E4, kind="ExternalInput").ap()
    ml_d = nc.dram_tensor("m_l", [D, D], F8E4, kind="ExternalInput").ap()
    nh_d = nc.dram_tensor("n_h", [D, D], F8E4, kind="ExternalInput").ap()
    nl_d = nc.dram_tensor("n_l", [D, D], F8E4, kind="ExternalInput").ap()
    mvq_d = nc.dram_tensor("mvq", [D, 1], F8E4, kind="ExternalInput").ap()
    mvk_d = nc.dram_tensor("mvk", [D, 1], F8E4, kind="ExternalInput").ap()
    c16_d = nc.dram_tensor("c16", [1, T], F8E4, kind="ExternalInput").ap()
    c1_d = nc.dram_tensor("c1", [1, T], F8E4, kind="ExternalInput").ap()
    bqbk_d = nc.dram_tensor("bqbk", [1, 1], F32, kind="ExternalInput").ap()
    bo2_d = nc.dram_tensor("bo2", [128, D], F32, kind="ExternalInput").ap()
    y_d = nc.dram_tensor("y", [T, D], F32, kind="ExternalOutput").ap()

    with tile.TileContext(nc) as tc:
        for _ in range(reps):
            _emit(tc, nc, xh_d, xl_d, mh_d, ml_d, nh_d, nl_d, mvq_d, mvk_d,
                  c16_d, c1_d, bqbk_d, bo2_d, y_d)
    nc.compile()


def _emit(tc, nc, xh_d, xl_d, mh_d, ml_d, nh_d, nl_d, mvq_d, mvk_d,
          c16_d, c1_d, bqbk_d, bo2_d, y_d):
    with (
        tc.tile_pool(name="const", bufs=1) as cp,
        tc.tile_pool(name="xw", bufs=1) as xp,
        tc.tile_pool(name="qu", bufs=1) as qp,
        tc.tile_pool(name="ex", bufs=2) as ep,
        tc.tile_pool(name="ps", bufs=4, space="PSUM") as pp,
        tc.tile_pool(name="yev", bufs=3) as yp,
    ):
        xh = xp.tile([128, ND, T], F8E4)
        xl = xp.tile([128, ND, T], F8E4)
        mh = xp.tile([128, ND, D], F8E4)
        ml = xp.tile([128, ND, D], F8E4)
        nh = xp.tile([128, ND, D], F8E4)
        nl = xp.tile([128, ND, D], F8E4)
        # 16 cols (value in col 0) so the DoubleRow pair stride is 16B-aligned
        mvq = cp.tile([128, ND, 16], F8E4)
        mvk = cp.tile([128, ND, 16], F8E4)
        bqbk = cp.tile([1, 1], F32)
        bo2 = cp.tile([128, D], F32)
        qext = cp.tile([1, 2, T], F8E4)   # [0]=qA (16x), [1]=16.0
        xext = cp.tile([1, 2, T], F8E4)   # [0]=1.0,      [1]=xB (1x)
        expb = cp.tile([128, 1], F32)     # exp bias: -CSH
        nc.vector.memset(expb[:], -CSH)

        qh = qp.tile([128, ND, T], F8E4)
        ql = qp.tile([128, ND, T], F8E4)
        uh = qp.tile([128, NT, UW], F8E4)
        ul = qp.tile([128, NT, UW], F8E4)

        # ---- input DMAs (first q'-proj group needs xh chunk 0 + mh) ----
        for n in range(NBLK):
            nc.sync.dma_start(
                xh[:, :, n * TQB:(n + 1) * TQB],
                xh_d[:, n * TQB:(n + 1) * TQB].rearrange("(j p) t -> p j t", p=128),
            )
        nc.sync.dma_start(mh[:], mh_d.rearrange("(j p) e -> p j e", p=128))
        for n in range(NBLK):
            nc.sync.dma_start(
                xl[:, :, n * TQB:(n + 1) * TQB],
                xl_d[:, n * TQB:(n + 1) * TQB].rearrange("(j p) t -> p j t", p=128),
            )
        nc.sync.dma_start(ml[:], ml_d.rearrange("(j p) e -> p j e", p=128))
        nc.sync.dma_start(mvq[:, :, 0:1], mvq_d.rearrange("(j p) o -> p j o", p=128))
        nc.sync.dma_start(mvk[:, :, 0:1], mvk_d.rearrange("(j p) o -> p j o", p=128))
        nc.sync.dma_start(bqbk[:], bqbk_d)
        nc.sync.dma_start(qext[:, 1, :], c16_d[:])
        nc.sync.dma_start(xext[:, 0, :], c1_d[:])
        nc.sync.dma_start(nh[:], nh_d.rearrange("(j p) e -> p j e", p=128))
        nc.sync.dma_start(nl[:], nl_d.rearrange("(j p) e -> p j e", p=128))
        nc.sync.dma_start(bo2[:], bo2_d)

        # denominator column (16.0) and its zero in ul
        nc.vector.memset(uh[:, :, D:D + 1], 16.0)
        nc.vector.memset(ul[:, :, D:D + 1], 0.0)

        # ---- bias matvecs: qA = xh@mvq + 16*bq.bk (16x), xB = (xh@mvk)/16 ----
        for n in range(NBLK):
            ps = pp.tile([1, TQB], F32, tag="mv", bufs=2)
            for j in range(ND // 2):
                nc.tensor.matmul(
                    ps[:], mvq[:, 2 * j:2 * j + 2, 0:1],
                    xh[:, 2 * j:2 * j + 2, n * TQB:(n + 1) * TQB],
                    start=(j == 0), stop=(j == ND // 2 - 1), perf_mode=DR,
                )
            nc.scalar.activation(qext[:, 0, n * TQB:(n + 1) * TQB], ps[:],
                                 AF.Identity, bias=bqbk[:, 0:1])
        for n in range(NBLK):
            ps = pp.tile([1, TQB], F32, tag="mv", bufs=2)
            for j in range(ND // 2):
                nc.tensor.matmul(
                    ps[:], mvk[:, 2 * j:2 * j + 2, 0:1],
                    xh[:, 2 * j:2 * j + 2, n * TQB:(n + 1) * TQB],
                    start=(j == 0), stop=(j == ND // 2 - 1), perf_mode=DR,
                )
            nc.scalar.activation(xext[:, 1, n * TQB:(n + 1) * TQB], ps[:],
                                 AF.Identity, scale=1.0 / 16.0)

        # ---- q' projection: PSUM = 16*(x@M) per (m, n); resplit hi/lo ----
        def emit_qproj(n):
            for m in range(ND):
                ps = pp.tile([128, TQB], F32, tag="ps")
                first = True
                for (mt, xt) in ((mh, xh), (mh, xl), (ml, xh)):
                    for j in range(ND // 2):
                        nc.tensor.matmul(
                            ps[:], mt[:, 2 * j:2 * j + 2, m * 128:(m + 1) * 128],
                            xt[:, 2 * j:2 * j + 2, n * TQB:(n + 1) * TQB],
                            start=first, stop=(mt is ml and j == ND // 2 - 1),
                            perf_mode=DR,
                        )
                        first = False
                sl = (slice(None), m, slice(n * TQB, (n + 1) * TQB))
                nc.scalar.activation(qh[sl], ps[:], AF.Identity)
                nc.vector.tensor_sub(ql[sl], ps[:], qh[sl])

        # ---- u projection: PSUM = 16*(x@N) per (i, ch); store at 16x ----
        def emit_uproj(i):
            for ch in range(2):
                ps = pp.tile([128, 384], F32, tag="ps")
                first = True
                for (xt, nt) in ((xh, nh), (xl, nh), (xh, nl)):
                    for j in range(ND // 2):
                        nc.tensor.matmul(
                            ps[:], xt[:, 2 * j:2 * j + 2, i * 128:(i + 1) * 128],
                            nt[:, 2 * j:2 * j + 2, ch * 384:(ch + 1) * 384],
                            start=first, stop=(xt is xh and nt is nl and j == ND // 2 - 1),
                            perf_mode=DR,
                        )
                        first = False
                sl = (slice(None), i, slice(ch * 384, (ch + 1) * 384))
                nc.scalar.activation(uh[sl], ps[:], AF.Identity)
                nc.vector.tensor_sub(ul[sl], ps[:], uh[sl])

        # ---- scores + exp for one block: PSUM = 128*s ----
        def emit_scores(blk, ehi, elo):
            tq = slice(blk * TQB, (blk + 1) * TQB)
            for i in range(NT):
                ps = pp.tile([128, TQB], F32, tag="ps")
                first = True
                for (xt, qt) in ((xh, qh), (xl, qh), (xh, ql)):
                    for j in range(ND // 2):
                        nc.tensor.matmul(
                            ps[:], xt[:, 2 * j:2 * j + 2, i * 128:(i + 1) * 128],
                            qt[:, 2 * j:2 * j + 2, tq],
                            start=first, stop=False, perf_mode=DR,
                        )
                        first = False
                nc.tensor.matmul(
                    ps[:], xext[:, :, i * 128:(i + 1) * 128], qext[:, :, tq],
                    start=False, stop=True, perf_mode=DR,
                )
                et = yp.tile([128, TQB], F32, tag="etmp", bufs=3)
                nc.scalar.activation(ehi[:, i, :], ps[:], AF.Exp,
                                     bias=expb[:], scale=1.0 / 128.0)
                nc.scalar.activation(et[:], ps[:], AF.Exp,
                                     bias=expb[:], scale=1.0 / 128.0)
                nc.vector.tensor_sub(elo[:, i, :], et[:], ehi[:, i, :])

        # ---- attention + output for one block: y = (e @ u) / den + bo2 ----
        def emit_attn(blk, ehi, elo):
            for l in range(TQB // 128):
                g = blk * (TQB // 128) + l
                tq = slice(l * 128, (l + 1) * 128)
                yt = yp.tile([128, D], F32, tag="yt")
                rc = yp.tile([128, 1], F32, tag="rc", bufs=2)
                for ch in (1, 0):  # denominator chunk first
                    lo = ch * 384
                    hi = D + 1 if ch == 1 else 384
                    ps = pp.tile([128, hi - lo], F32, tag="ys", bufs=2)
                    first = True
                    for (et, ut) in ((ehi, uh), (elo, uh), ((ehi, ul))):
                        for i in range(NT // 2):
                            nc.tensor.matmul(
                                ps[:], et[:, 2 * i:2 * i + 2, tq],
                                ut[:, 2 * i:2 * i + 2, lo:hi],
                                start=first, stop=(et is ehi and ut is ul and i == NT // 2 - 1),
                                perf_mode=DR,
                            )
                            first = False
                    if ch == 1:
                        nc.vector.reciprocal(rc[:], ps[:, D - lo:D - lo + 1])
                        nc.vector.scalar_tensor_tensor(
                            yt[:, lo:D], ps[:, 0:D - lo], rc[:], bo2[:, lo:D],
                            op0=mybir.AluOpType.mult, op1=mybir.AluOpType.add,
                        )
                    else:
                        nc.vector.scalar_tensor_tensor(
                            yt[:, lo:384], ps[:], rc[:], bo2[:, lo:384],
                            op0=mybir.AluOpType.mult, op1=mybir.AluOpType.add,
                        )
                nc.sync.dma_start(y_d[g * 128:(g + 1) * 128, :], yt[:])

        # ---- schedule ----
        eb = [(ep.tile([128, NT, TQB], F8E5, tag="ehi", name=f"eh{p}"),
               ep.tile([128, NT, TQB], F8E4, tag="elo", name=f"el{p}"))
              for p in range(2)]

        emit_qproj(0)
        emit_scores(0, *eb[0])
        for n in range(1, NBLK):
            emit_qproj(n)
        for i in range(NT):
            emit_uproj(i)
        emit_scores(1, *eb[1])
        emit_attn(0, *eb[0])
        emit_scores(2, *eb[0])
        emit_attn(1, *eb[1])
        emit_scores(3, *eb[1])
        emit_attn(2, *eb[0])
        emit_attn(3, *eb[1])


_NC_CACHE = None


def build_nc(reps=1):
    nc = bacc.Bacc("TRN2", target_bir_lowering=False, debug=False)
    _build_program(nc, reps=reps)
    return nc


def _get_nc():
    global _NC_CACHE
    if _NC_CACHE is None:
        _NC_CACHE = build_nc(1)
    return _NC_CACHE


def _q4(a):
    return np.clip(a, -240.0, 240.0).astype(E4NP)


def host_prep(x, w_qkv, b_qkv, w_out, b_out):
    """Host-side folding + fp8 hi/lo splits. Returns (shared dict, per-core xT list)."""
    x = np.asarray(x, np.float32)
    w_qkv = np.asarray(w_qkv, np.float32)
    b_qkv = np.asarray(b_qkv, np.float32)
    w_out = np.asarray(w_out, np.float32)
    b_out = np.asarray(b_out, np.float32)

    wq, wk, wv = w_qkv[:, :D], w_qkv[:, D:2 * D], w_qkv[:, 2 * D:]
    bq, bk, bv = b_qkv[:D], b_qkv[D:2 * D], b_qkv[2 * D:]
    M = wq @ wk.T
    N = wv @ w_out
    mh = _q4(16.0 * M)
    ml = _q4(16.0 * M - mh.astype(np.float32))
    nh = _q4(16.0 * N)
    nl = _q4(16.0 * N - nh.astype(np.float32))
    shared = {
        "m_h": mh, "m_l": ml, "n_h": nh, "n_l": nl,
        "mvq": _q4(16.0 * (wq @ bk)).reshape(D, 1),
        "mvk": _q4(16.0 * (wk @ bq)).reshape(D, 1),
        "c16": np.full((1, T), 16.0, E4NP),
        "c1": np.full((1, T), 1.0, E4NP),
        "bqbk": np.full((1, 1), 16.0 * float(bq @ bk), np.float32),
        "bo2": np.ascontiguousarray(
            np.broadcast_to((b_out + bv @ w_out).reshape(1, D), (128, D))),
    }
    xts = []
    for c in range(x.shape[0]):
        xT = np.ascontiguousarray(x[c].T)
        xth = _q4(xT)
        xtl = _q4(xT - xth.astype(np.float32))
        xts.append((xth, xtl))
    return shared, xts


def kernel(x, w_qkv, b_qkv, w_out, b_out):
    shared, xts = host_prep(x, w_qkv, b_qkv, w_out, b_out)
    nc = _get_nc()
    in_maps = [{**shared, "xt_h": xts[c][0], "xt_l": xts[c][1]} for c in range(B)]
    try:
        res = bass_utils.run_bass_kernel_spmd(nc, in_maps, core_ids=list(range(B)))
    except Exception:
        res = bass_utils.run_bass_kernel_spmd(nc, in_maps, core_ids=list(range(B)))
    return np.stack([res.results[c]["y"] for c in range(B)], axis=0)


# revision 14
# speedup vs baseline: 1.7443x; 1.7443x over previous
"""Fused MHA Bass kernel for Trainium2, batch-parallel over 8 cores, bf16.

Reference (per batch element):
    qkv = x @ w_qkv + b_qkv ; q,k,v = split(qkv)
    s = q @ k.T / 8 ; a = softmax(s) ; y = (a @ v) @ w_out + b_out

Structural folding (exact algebra, host-side weight fusion):
    s*8 = x M x^T + 1 (x wk bq)^T + [per-row terms],   M = wq wk^T
    y   = (a_unnorm @ u) / den + (b_out + bv w_out),   u = x (wv w_out)
- The k/v projections and the output projection collapse into two [768,768]
  device matmuls (q' = x@M, u = x@N).
- The per-row (tq) score bias terms cancel under softmax shift-invariance
  and are simply dropped; the per-column (tk) term x@(wk bq) is a
  per-partition bias folded into the q'-eviction for free.
- Attention output is computed directly in [tq, dy] layout (exps stationary),
  so the softmax denominator is an appended ones-column of u, landing
  per-partition: one DVE reciprocal, no transposes anywhere, and the
  normalization + output bias fuse into the PSUM->SBUF y eviction.
Everything runs in bf16 (same PE rate as f32r, half the SBUF/DMA): all
tensors SBUF-resident, no DRAM spills, zero collectives.
Measured rel err vs fp32 reference: ~1.0e-2 (gate 2e-2).

Per-core PE work: q'-proj 73.7k + u-proj 73.7k + scores 196.6k +
attn 196.9k = 541k PE columns (baseline structure: 688k).
"""

import numpy as np
import ml_dtypes

import concourse.bacc as bacc
import concourse.bass as bass
import concourse.mybir as mybir
import concourse.tile as tile
from concourse import bass_utils

F32 = mybir.dt.float32
BF16 = mybir.dt.bfloat16
AF = mybir.ActivationFunctionType

B = 8
T = 2048
D = 768
ND = D // 128           # 6 d-tiles
NT = T // 128           # 16 t-tiles
TQB = 512               # query-block width
NBLK = T // TQB         # 4 blocks
UW = D + 8              # u width: col D = 1.0 denominator column, rest pad
BFNP = ml_dtypes.bfloat16


def _build_program(nc, reps=1):
    x_d = nc.dram_tensor("xt_bf", [D, T], BF16, kind="ExternalInput").ap()
    m_d = nc.dram_tensor("m_bf", [D, D], BF16, kind="ExternalInput").ap()
    n_d = nc.dram_tensor("n_bf", [D, D], BF16, kind="ExternalInput").ap()
    mvk_d = nc.dram_tensor("mvkt", [128, ND], F32, kind="ExternalInput").ap()
    bo2_d = nc.dram_tensor("bo2", [128, D], F32, kind="ExternalInput").ap()
    y_d = nc.dram_tensor("y", [T, D], F32, kind="ExternalOutput").ap()

    with tile.TileContext(nc) as tc:
        for _ in range(reps):
            _emit(tc, nc, x_d, m_d, n_d, mvk_d, bo2_d, y_d)
    nc.compile()


def _emit(tc, nc, x_d, m_d, n_d, mvk_d, bo2_d, y_d):
    with (
        tc.tile_pool(name="const", bufs=1) as cp,
        tc.tile_pool(name="xw", bufs=1) as xp,
        tc.tile_pool(name="qu", bufs=1) as qp,
        tc.tile_pool(name="ex", bufs=2) as ep,
        tc.tile_pool(name="ps", bufs=4, space="PSUM") as pp,
        tc.tile_pool(name="yev", bufs=3) as yp,
    ):
        xbf = xp.tile([128, ND, T], BF16)
        mbf = xp.tile([128, ND, D], BF16)
        nbf = xp.tile([128, ND, D], BF16)
        mvkt = cp.tile([128, ND], F32)
        bo2 = cp.tile([128, D], F32)
        qbf = qp.tile([128, ND, T], BF16)
        ubf = qp.tile([128, NT, UW], BF16)

        # ---- input DMAs: first q'-proj group needs xbf chunk 0 + mbf ----
        for n in range(NBLK):
            nc.sync.dma_start(
                xbf[:, :, n * TQB:(n + 1) * TQB],
                x_d[:, n * TQB:(n + 1) * TQB].rearrange("(j p) t -> p j t", p=128),
            )
        nc.sync.dma_start(mbf[:], m_d.rearrange("(j p) e -> p j e", p=128))
        nc.sync.dma_start(mvkt[:], mvk_d)
        nc.sync.dma_start(nbf[:], n_d.rearrange("(j p) e -> p j e", p=128))
        nc.sync.dma_start(bo2[:], bo2_d)
        nc.vector.memset(ubf[:, :, D:D + 1], 1.0)  # denominator column

        def emit_qproj(n):
            # PSUM = (x@M)[e-tile m, t-chunk n]; evict bf16 + per-e bias (x wk bq)
            for m in range(ND):
                ps = pp.tile([128, TQB], F32, tag="ps")
                for j in range(ND):
                    nc.tensor.matmul(
                        ps[:], mbf[:, j, m * 128:(m + 1) * 128],
                        xbf[:, j, n * TQB:(n + 1) * TQB],
                        start=(j == 0), stop=(j == ND - 1),
                    )
                nc.scalar.activation(qbf[:, m, n * TQB:(n + 1) * TQB], ps[:],
                                     AF.Identity, bias=mvkt[:, m:m + 1])

        def emit_uproj(i):
            for ch in range(2):
                ps = pp.tile([128, 384], F32, tag="ps")
                for j in range(ND):
                    nc.tensor.matmul(
                        ps[:], xbf[:, j, i * 128:(i + 1) * 128],
                        nbf[:, j, ch * 384:(ch + 1) * 384],
                        start=(j == 0), stop=(j == ND - 1),
                    )
                nc.scalar.activation(ubf[:, i, ch * 384:(ch + 1) * 384], ps[:],
                                     AF.Identity)

        def emit_scores(blk, eb):
            # scores^T tile [tk, tq]; exp(s/8 [+ per-tk bias]) fused into eviction
            tq = slice(blk * TQB, (blk + 1) * TQB)
            for i in range(NT):
                ps = pp.tile([128, TQB], F32, tag="ps")
                for j in range(ND):
                    nc.tensor.matmul(
                        ps[:], xbf[:, j, i * 128:(i + 1) * 128], qbf[:, j, tq],
                        start=(j == 0), stop=(j == ND - 1),
                    )
                nc.scalar.activation(eb[:, i, :], ps[:], AF.Exp, scale=0.125)

        def emit_attn(blk, eb):
            # y[tq, dy] = (e @ u) * recip + bo2, denominator from u's ones-column
            for l in range(TQB // 128):
                g = blk * (TQB // 128) + l
                tq = slice(l * 128, (l + 1) * 128)
                yt = yp.tile([128, D], F32, tag="yt")
                rc = yp.tile([128, 1], F32, tag="rc", bufs=2)
                for ch in (1, 0):  # denominator chunk first
                    lo = ch * 384
                    hi = D + 1 if ch == 1 else 384
                    ps = pp.tile([128, hi - lo], F32, tag="ys", bufs=2)
                    for i in range(NT):
                        nc.tensor.matmul(
                            ps[:], eb[:, i, tq], ubf[:, i, lo:hi],
                            start=(i == 0), stop=(i == NT - 1),
                        )
                    if ch == 1:
                        nc.vector.reciprocal(rc[:], ps[:, D - lo:D - lo + 1])
                        nc.vector.scalar_tensor_tensor(
                            yt[:, lo:D], ps[:, 0:D - lo], rc[:], bo2[:, lo:D],
                            op0=mybir.AluOpType.mult, op1=mybir.AluOpType.add,
                        )
                    else:
                        nc.vector.scalar_tensor_tensor(
                            yt[:, lo:384], ps[:], rc[:], bo2[:, lo:384],
                            op0=mybir.AluOpType.mult, op1=mybir.AluOpType.add,
                        )
                nc.sync.dma_start(y_d[g * 128:(g + 1) * 128, :], yt[:])

        # ---- schedule: interleave so exp/DVE evictions hide under PE ----
        eb = [ep.tile([128, NT, TQB], BF16, tag="ebf", name=f"eb{p}")
              for p in range(2)]

        emit_qproj(0)
        emit_scores(0, eb[0])
        for n in range(1, NBLK):
            emit_qproj(n)
        for i in range(NT):
            emit_uproj(i)
        emit_scores(1, eb[1])
        emit_attn(0, eb[0])
        emit_scores(2, eb[0])
        emit_attn(1, eb[1])
        emit_scores(3, eb[1])
        emit_attn(2, eb[0])
        emit_attn(3, eb[1])


_NC_CACHE = None


def build_nc(reps=1):
    nc = bacc.Bacc("TRN2", target_bir_lowering=False, debug=False)
    _build_program(nc, reps=reps)
    return nc


def _get_nc():
    global _NC_CACHE
    if _NC_CACHE is None:
        _NC_CACHE = build_nc(1)
    return _NC_CACHE


def host_prep(x, w_qkv, b_qkv, w_out, b_out):
    """Host-side weight folding. Returns (shared input dict, per-core xT list)."""
    x = np.asarray(x, np.float32)
    w_qkv = np.asarray(w_qkv, np.float32)
    b_qkv = np.asarray(b_qkv, np.float32)
    w_out = np.asarray(w_out, np.float32)
    b_out = np.asarray(b_out, np.float32)

    wq, wk, wv = w_qkv[:, :D], w_qkv[:, D:2 * D], w_qkv[:, 2 * D:]
    bq, bk, bv = b_qkv[:D], b_qkv[D:2 * D], b_qkv[2 * D:]
    shared = {
        "m_bf": (wq @ wk.T).astype(BFNP),
        "n_bf": (wv @ w_out).astype(BFNP),
        "mvkt": np.ascontiguousarray((wk @ bq).reshape(ND, 128).T.astype(np.float32)),
        "bo2": np.ascontiguousarray(
            np.broadcast_to((b_out + bv @ w_out).reshape(1, D), (128, D))),
    }
    xts = [np.ascontiguousarray(x[c].T).astype(BFNP) for c in range(x.shape[0])]
    return shared, xts


def kernel(x, w_qkv, b_qkv, w_out, b_out):
    shared, xts = host_prep(x, w_qkv, b_qkv, w_out, b_out)
    nc = _get_nc()
    in_maps = [{**shared, "xt_bf": xts[c]} for c in range(B)]
    try:
        res = bass_utils.run_bass_kernel_spmd(nc, in_maps, core_ids=list(range(B)))
    except Exception:
        res = bass_utils.run_bass_kernel_spmd(nc, in_maps, core_ids=list(range(B)))
    return np.stack([res.results[c]["y"] for c in range(B)], axis=0)
